# revision 1
# baseline (speedup 1.0000x reference)
"""Trainium2 Bass kernel for nn_Bottleneck_5669356834470 (ResNet bottleneck
with an involution middle layer).

Sharding: data-parallel over batch. 16 samples / 8 cores = 2 samples/core.
All weights replicated (tiny).

Per-core pipeline (spatial 56x56 = 3136 flattened, S=2 samples):
  conv1 (1x1, 256->64) +BN1+ReLU   : PE matmuls (bf16), ACT evac w/ fused
                                     scale(folded)+bias+relu
  inv_c1 (1x1, 64->16) +BN+ReLU    : PE, ACT evac
  inv_c2 (1x1, 16->196) + bias     : PE, ACT evac -> dynamic weights w'
  involution (G=4, 7x7 dynamic)    : DVE tensor_tensor ops in a
      (sample, group, 4-row-chunk) partition layout (112 partitions); dynamic
      weights broadcast across the 16 group channels via a 0-stride AP dim;
      spatial shifts are free-dim slices of a zero-padded halo tensor.
  BN2+ReLU                         : ACT
  conv3 (1x1, 64->256) +BN3 + residual + ReLU : PE (residual folded in as an
      identity matmul over bf16 input), ACT evac w/ fused bias+relu.

Activation layout trick: activations are stored [channels, (sample, space)] so
every matmul operand sits at partition base 0 (no tile_position needed).

Compute dtype bf16 (f32 PSUM accumulation); output f32.
"""

import sys

sys.path.insert(0, "/opt/trn_rl_repo")

import numpy as np
import ml_dtypes

BF16 = ml_dtypes.bfloat16

S = 2            # samples per core
N_CORES = 8
CIN = 256
CMID = 64
G = 4            # involution groups
GC = 16          # channels per group
KS = 7           # involution kernel size
KK = KS * KS     # 49
R = 16           # dyn-weight bottleneck channels
H = W = 56
HW = H * W       # 3136
NCH = 7          # spatial chunks for matmul N dim
NW = HW // NCH   # 448
M = 14           # 4-row chunks per (sample, group)
RH = 4           # output rows per chunk
HR = 10          # halo rows stored per chunk (-3..+6)
WP = 62          # padded row width
NP_INV = S * G * M          # 112 involution partitions
XUF = GC * HR * W           # 8960 free elems per XU partition
XHF = GC * HR * WP          # 9920 free elems per XH partition
W2F = KK * RH * W           # 10976 free elems per W2 partition
ACCF = GC * RH * W          # 3584 acc free elems per partition
EPS = 1e-5

_CACHE = {}


def _ap(tile_ap, off, dims):
    """Raw strided AP on a tile's underlying tensor. dims=[(step,count),...]
    in elements; for SBUF the partition stride is ap[0][0] of the base AP."""
    import bass_rust

    return bass_rust.AP(tile_ap.tensor, tile_ap.offset + off, [list(d) for d in dims])


def build_module():
    if "nc" in _CACHE:
        return _CACHE["nc"]
    import concourse.bacc as bacc
    import concourse.mybir as mybir
    import concourse.tile as tile

    dt = mybir.dt
    AF = mybir.ActivationFunctionType

    nc = bacc.Bacc("TRN2", debug=False, num_devices=N_CORES)

    # ---- DRAM I/O ----------------------------------------------------------
    xin = nc.dram_tensor("xin", [S, CIN, HW], dt.float32, kind="ExternalInput")
    w1t = nc.dram_tensor("w1t", [2, 128, CMID], dt.bfloat16, kind="ExternalInput")
    b1 = nc.dram_tensor("b1", [CMID, 1], dt.float32, kind="ExternalInput")
    c1t = nc.dram_tensor("c1t", [CMID, R], dt.bfloat16, kind="ExternalInput")
    bi = nc.dram_tensor("bi", [R, 1], dt.float32, kind="ExternalInput")
    c2t = nc.dram_tensor("c2t", [R, G * KK], dt.bfloat16, kind="ExternalInput")
    b2ca = nc.dram_tensor("b2ca", [128, 1], dt.float32, kind="ExternalInput")
    b2cb = nc.dram_tensor("b2cb", [68, 1], dt.float32, kind="ExternalInput")
    s2v = nc.dram_tensor("s2v", [CMID, 1], dt.float32, kind="ExternalInput")
    b2v = nc.dram_tensor("b2v", [CMID, 1], dt.float32, kind="ExternalInput")
    w3t = nc.dram_tensor("w3t", [2, CMID, 128], dt.bfloat16, kind="ExternalInput")
    b3 = nc.dram_tensor("b3", [128, 2], dt.float32, kind="ExternalInput")
    ident = nc.dram_tensor("ident", [128, 128], dt.bfloat16, kind="ExternalInput")
    out = nc.dram_tensor("out", [S, CIN, HW], dt.float32, kind="ExternalOutput")

    with tile.TileContext(nc) as tc:
        with (
            tc.tile_pool(name="consts", bufs=1) as cpool,
            tc.tile_pool(name="big", bufs=1) as bpool,
            tc.tile_pool(name="psum", bufs=6, space="PSUM") as ppool,
            tc.tile_pool(name="stage", bufs=4) as spool,
            tc.tile_pool(name="dstage", bufs=1, space="DRAM") as dpool,
        ):
            # ---- constants -> SBUF ----------------------------------------
            w1t_sb = cpool.tile([128, 2 * CMID], dt.bfloat16, tag="w1t")
            nc.sync.dma_start(
                out=w1t_sb[:, :].rearrange("p (k c) -> p k c", k=2),
                in_=w1t.ap().rearrange("k p c -> p k c"),
            )
            b1_sb = cpool.tile([CMID, 1], dt.float32, tag="b1")
            nc.sync.dma_start(out=b1_sb[:, :], in_=b1.ap())
            c1t_sb = cpool.tile([CMID, R], dt.bfloat16, tag="c1t")
            nc.sync.dma_start(out=c1t_sb[:, :], in_=c1t.ap())
            bi_sb = cpool.tile([R, 1], dt.float32, tag="bi")
            nc.sync.dma_start(out=bi_sb[:, :], in_=bi.ap())
            c2t_sb = cpool.tile([R, G * KK], dt.bfloat16, tag="c2t")
            nc.sync.dma_start(out=c2t_sb[:, :], in_=c2t.ap())
            b2ca_sb = cpool.tile([128, 1], dt.float32, tag="b2ca")
            nc.sync.dma_start(out=b2ca_sb[:, :], in_=b2ca.ap())
            b2cb_sb = cpool.tile([68, 1], dt.float32, tag="b2cb")
            nc.sync.dma_start(out=b2cb_sb[:, :], in_=b2cb.ap())
            s2v_sb = cpool.tile([CMID, 1], dt.float32, tag="s2v")
            nc.sync.dma_start(out=s2v_sb[:, :], in_=s2v.ap())
            b2v_sb = cpool.tile([CMID, 1], dt.float32, tag="b2v")
            nc.sync.dma_start(out=b2v_sb[:, :], in_=b2v.ap())
            w3t_sb = cpool.tile([CMID, 2 * 128], dt.bfloat16, tag="w3t")
            nc.sync.dma_start(
                out=w3t_sb[:, :].rearrange("p (k c) -> p k c", k=2),
                in_=w3t.ap().rearrange("k p c -> p k c"),
            )
            b3_sb = cpool.tile([128, 2], dt.float32, tag="b3")
            nc.sync.dma_start(out=b3_sb[:, :], in_=b3.ap())
            id_sb = cpool.tile([128, 128], dt.bfloat16, tag="ident")
            nc.sync.dma_start(out=id_sb[:, :], in_=ident.ap())

            # ---- x load (f32 -> bf16 cast during SWDGE DMA) ---------------
            xbf = bpool.tile([128, S * 2 * HW], dt.bfloat16, tag="xbf")
            xbf_v = xbf[:, :].rearrange("p (s k f) -> p s k f", s=S, k=2)
            nc.gpsimd.dma_start(
                out=xbf_v,
                in_=xin.ap().rearrange("s (k p) f -> p s k f", p=128),
            )

            # ---- conv1 + BN1 + ReLU  -> out1 [64, (s, hw)] bf16 -----------
            out1 = bpool.tile([CMID, S * HW], dt.bfloat16, tag="out1")
            w1t_v = w1t_sb[:, :].rearrange("p (k c) -> p k c", k=2)
            for s in range(S):
                for n in range(NCH):
                    ps = ppool.tile([128, NW], dt.float32, tag="ps")
                    for kc in range(2):
                        nc.tensor.matmul(
                            ps[:CMID, :],
                            w1t_v[:, kc, :],
                            xbf_v[:, s, kc, n * NW : (n + 1) * NW],
                            start=(kc == 0),
                            stop=(kc == 1),
                        )
                    nc.scalar.activation(
                        out1[:, s * HW + n * NW : s * HW + (n + 1) * NW],
                        ps[:CMID, :],
                        AF.Relu,
                        bias=b1_sb[:, 0:1],
                    )

            # ---- inv_c1 + BN + ReLU -> z [16, (s, hw)] bf16 ---------------
            z_sb = bpool.tile([R, S * HW], dt.bfloat16, tag="z")
            for s in range(S):
                for n in range(NCH):
                    ps = ppool.tile([128, NW], dt.float32, tag="ps")
                    nc.tensor.matmul(
                        ps[:R, :],
                        c1t_sb[:, :],
                        out1[:, s * HW + n * NW : s * HW + (n + 1) * NW],
                        start=True,
                        stop=True,
                    )
                    nc.scalar.activation(
                        z_sb[:, s * HW + n * NW : s * HW + (n + 1) * NW],
                        ps[:R, :],
                        AF.Relu,
                        bias=bi_sb[:, 0:1],
                    )

            # ---- inv_c2 + bias -> w2a [128,(s,hw)], w2b [68,(s,hw)] -------
            w2a = bpool.tile([128, S * HW], dt.bfloat16, tag="w2a")
            w2b = bpool.tile([68, S * HW], dt.bfloat16, tag="w2b")
            for s in range(S):
                for n in range(NCH):
                    sl = slice(s * HW + n * NW, s * HW + (n + 1) * NW)
                    psa = ppool.tile([128, NW], dt.float32, tag="ps")
                    psb = ppool.tile([128, NW], dt.float32, tag="ps")
                    nc.tensor.matmul(
                        psa[:, :],
                        c2t_sb[:, 0:128],
                        z_sb[:, sl],
                        start=True,
                        stop=True,
                    )
                    nc.tensor.matmul(
                        psb[:68, :],
                        c2t_sb[:, 128:196],
                        z_sb[:, sl],
                        start=True,
                        stop=True,
                    )
                    nc.scalar.activation(
                        w2a[:, sl], psa[:, :], AF.Identity, bias=b2ca_sb[:, 0:1]
                    )
                    nc.scalar.activation(
                        w2b[:, sl], psb[:68, :], AF.Identity, bias=b2cb_sb[:, 0:1]
                    )

            # ---- involution operand builds --------------------------------
            # XU: unpadded halo rows, (s,g,m) partition layout
            xu = bpool.tile([NP_INV, XUF], dt.bfloat16, tag="xu")
            xh = bpool.tile([NP_INV, XHF], dt.bfloat16, tag="xh")
            xh2 = bpool.tile([NP_INV, XHF], dt.bfloat16, tag="xh2")
            w2t = bpool.tile([NP_INV, W2F], dt.bfloat16, tag="w2t")
            nc.any.memset(xu[:, :], 0.0)
            nc.any.memset(xh[:, :], 0.0)
            nc.any.memset(xh2[:, :], 0.0)

            # SBUF->SBUF DMAs cannot transpose the partition dim, so bounce
            # the layout changes through DRAM staging tiles (dep-tracked).
            out1d = dpool.tile([CMID, S * HW], dt.bfloat16, tag="out1d")
            nc.sync.dma_start(out=out1d[:, :], in_=out1[:, :])
            xu_ap = xu[:, :]
            o1d_ap = out1d[:, :]
            P_XU = xu_ap.ap[0][0]
            D_O1 = S * HW
            for s in range(S):
                for g in range(G):
                    pb = (s * G + g) * M
                    cb = g * GC
                    # middle chunks m=1..12: all 10 halo rows valid.
                    # iteration (m, c, run): dst partition dim first.
                    nc.sync.dma_start(
                        out=_ap(
                            xu_ap,
                            (pb + 1) * P_XU,
                            [(P_XU, 12), (HR * W, GC), (1, HR * W)],
                        ),
                        in_=_ap(
                            o1d_ap,
                            cb * D_O1 + s * HW + 1 * W,
                            [(RH * W, 12), (D_O1, GC), (1, HR * W)],
                        ),
                    )
                    # m=0: rows 0..6 -> r=3..9
                    nc.sync.dma_start(
                        out=_ap(
                            xu_ap,
                            pb * P_XU + 3 * W,
                            [(P_XU, 1), (HR * W, GC), (1, 7 * W)],
                        ),
                        in_=_ap(
                            o1d_ap,
                            cb * D_O1 + s * HW,
                            [(RH * W, 1), (D_O1, GC), (1, 7 * W)],
                        ),
                    )
                    # m=13: rows 49..55 -> r=0..6
                    nc.sync.dma_start(
                        out=_ap(
                            xu_ap,
                            (pb + 13) * P_XU,
                            [(P_XU, 1), (HR * W, GC), (1, 7 * W)],
                        ),
                        in_=_ap(
                            o1d_ap,
                            cb * D_O1 + s * HW + 49 * W,
                            [(RH * W, 1), (D_O1, GC), (1, 7 * W)],
                        ),
                    )

            # expand XU (56-wide rows) into XH (62-wide zero-padded rows) and
            # XH2 (same, shifted right one element for 4B-aligned odd taps)
            xu_v = xu[:, :].rearrange("p (c r w) -> p c r w", r=HR, w=W)
            xh_v = xh[:, :].rearrange("p (c r w) -> p c r w", r=HR, w=WP)
            xh2_v = xh2[:, :].rearrange("p (c r w) -> p c r w", r=HR, w=WP)
            nc.vector.tensor_copy(xh_v[:, :, :, 3 : 3 + W], xu_v)
            nc.vector.tensor_copy(xh2_v[:, :, :, 4 : 4 + W], xu_v)

            # W2: dynamic weights in (s,g,m) layout, free = (k, rh, w);
            # staged via DRAM (w2d) to transpose the partition dim.
            w2d = dpool.tile([G * KK, S * HW], dt.bfloat16, tag="w2d")
            nc.sync.dma_start(out=w2d[0:128, :], in_=w2a[:, :])
            nc.sync.dma_start(out=w2d[128 : G * KK, :], in_=w2b[:, :])
            w2t_ap = w2t[:, :]
            w2d_ap = w2d[:, :]
            P_W2 = w2t_ap.ap[0][0]
            D_W2 = S * HW
            for s in range(S):
                for g in range(G):
                    pb = (s * G + g) * M
                    # iteration (m, k, run): dst partition dim first
                    nc.sync.dma_start(
                        out=_ap(
                            w2t_ap,
                            pb * P_W2,
                            [(P_W2, M), (RH * W, KK), (1, RH * W)],
                        ),
                        in_=_ap(
                            w2d_ap,
                            (g * KK) * D_W2 + s * HW,
                            [(RH * W, M), (D_W2, KK), (1, RH * W)],
                        ),
                    )

            # ---- involution: 49 taps of mul + accumulate on DVE -----------
            acc = bpool.tile([NP_INV, ACCF], dt.bfloat16, tag="acc")
            tmp = bpool.tile([NP_INV, ACCF], dt.bfloat16, tag="tmp")
            acc_v = acc[:, :].rearrange("p (c r w) -> p c r w", r=RH, w=W)
            tmp_v = tmp[:, :].rearrange("p (c r w) -> p c r w", r=RH, w=W)
            w2t_v = w2t[:, :].rearrange("p (k r w) -> p k r w", k=KK, r=RH)
            for k in range(KK):
                kh, kw = divmod(k, KS)
                if kw % 2 == 0:
                    src_v = xh_v          # offset kh*62+kw even -> 4B aligned
                    wc = kw
                else:
                    src_v = xh2_v         # XH2[w'] = XH[w'-1]; kw+1 even
                    wc = kw + 1
                in0 = src_v[:, :, kh : kh + RH, wc : wc + W]
                in1 = w2t_v[:, k : k + 1, :, :].to_broadcast([NP_INV, GC, RH, W])
                if k == 0:
                    nc.vector.tensor_mul(acc_v, in0, in1)
                else:
                    nc.vector.tensor_mul(tmp_v, in0, in1)
                    nc.vector.tensor_add(acc_v, acc_v, tmp_v)

            # ---- ACC -> (DRAM) -> out2 [64, (s, hw)], BN2+ReLU ------------
            accd = dpool.tile([NP_INV, ACCF], dt.bfloat16, tag="accd")
            nc.sync.dma_start(out=accd[:, :], in_=acc[:, :])
            out2 = bpool.tile([CMID, S * HW], dt.bfloat16, tag="out2")
            o2_ap = out2[:, :]
            P_O2 = o2_ap.ap[0][0]
            acd_ap = accd[:, :]
            for s in range(S):
                for g in range(G):
                    pb = (s * G + g) * M
                    # iteration (c, m, run): dst partition dim first
                    nc.sync.dma_start(
                        out=_ap(
                            o2_ap,
                            (g * GC) * P_O2 + s * HW,
                            [(P_O2, GC), (RH * W, M), (1, RH * W)],
                        ),
                        in_=_ap(
                            acd_ap,
                            pb * ACCF,
                            [(RH * W, GC), (ACCF, M), (1, RH * W)],
                        ),
                    )
            relu2 = bpool.tile([CMID, S * HW], dt.bfloat16, tag="relu2")
            for s in range(S):
                nc.scalar.activation(
                    relu2[:, s * HW : (s + 1) * HW],
                    out2[:, s * HW : (s + 1) * HW],
                    AF.Relu,
                    bias=b2v_sb[:, 0:1],
                    scale=s2v_sb[:, 0:1],
                )

            # ---- conv3 + BN3 + residual + ReLU -> out ---------------------
            w3t_v = w3t_sb[:, :].rearrange("p (k c) -> p k c", k=2)
            for s in range(S):
                for oc in range(2):
                    for n in range(NCH):
                        ps = ppool.tile([128, NW], dt.float32, tag="ps")
                        nc.tensor.matmul(
                            ps[:, :],
                            w3t_v[:, oc, :],
                            relu2[:, s * HW + n * NW : s * HW + (n + 1) * NW],
                            start=True,
                            stop=False,
                        )
                        nc.tensor.matmul(
                            ps[:, :],
                            id_sb[:, :],
                            xbf_v[:, s, oc, n * NW : (n + 1) * NW],
                            start=False,
                            stop=True,
                        )
                        ob = spool.tile([128, NW], dt.float32, tag="obuf")
                        nc.scalar.activation(
                            ob[:, :], ps[:, :], AF.Relu, bias=b3_sb[:, oc : oc + 1]
                        )
                        nc.sync.dma_start(
                            out=out.ap()[
                                s, oc * 128 : (oc + 1) * 128, n * NW : (n + 1) * NW
                            ],
                            in_=ob[:, :],
                        )

    nc.compile()
    _CACHE["nc"] = nc
    return nc


def _f32(a):
    return np.ascontiguousarray(a, dtype=np.float32)


def prep_weights(inputs):
    """Host-side folding of BN scales into conv weights; bf16 casts."""
    f = inputs
    s1 = f["bn1_g"] / np.sqrt(f["bn1_v"] + EPS)
    b1_eff = f["bn1_b"] - f["bn1_m"] * s1
    w1t_eff = (_f32(f["conv1_w"]) * s1[:, None]).T          # [256, 64]

    si = f["inv_bn_g"] / np.sqrt(f["inv_bn_v"] + EPS)
    bi_eff = f["inv_bn_b"] - f["inv_bn_m"] * si
    c1t_eff = (_f32(f["inv_c1_w"]) * si[:, None]).T         # [64, 16]

    c2t_eff = _f32(f["inv_c2_w"]).T                         # [16, 196]
    b2c = _f32(f["inv_c2_b"])

    s2 = f["bn2_g"] / np.sqrt(f["bn2_v"] + EPS)
    b2n = f["bn2_b"] - f["bn2_m"] * s2

    s3 = f["bn3_g"] / np.sqrt(f["bn3_v"] + EPS)
    b3_eff = f["bn3_b"] - f["bn3_m"] * s3
    w3t_eff = (_f32(f["conv3_w"]) * s3[:, None]).T          # [64, 256]

    d = {}
    d["w1t"] = np.ascontiguousarray(
        w1t_eff.reshape(2, 128, CMID).astype(BF16)
    )
    d["b1"] = _f32(b1_eff)[:, None]
    d["c1t"] = np.ascontiguousarray(c1t_eff.astype(BF16))
    d["bi"] = _f32(bi_eff)[:, None]
    d["c2t"] = np.ascontiguousarray(c2t_eff.astype(BF16))
    d["b2ca"] = _f32(b2c[0:128])[:, None]
    d["b2cb"] = _f32(b2c[128:196])[:, None]
    d["s2v"] = _f32(s2)[:, None]
    d["b2v"] = _f32(b2n)[:, None]
    d["w3t"] = np.ascontiguousarray(
        w3t_eff.reshape(CMID, 2, 128).transpose(1, 0, 2).astype(BF16)
    )
    d["b3"] = _f32(b3_eff.reshape(2, 128).T)
    d["ident"] = np.ascontiguousarray(np.eye(128, dtype=np.float32).astype(BF16))
    return d


def make_in_maps(inputs):
    prep = prep_weights(inputs)
    x = _f32(inputs["x"]).reshape(16, CIN, HW)
    in_maps = []
    for i in range(N_CORES):
        m = dict(prep)
        m["xin"] = np.ascontiguousarray(x[S * i : S * i + S])
        in_maps.append(m)
    return in_maps


def kernel(**inputs):
    from concourse.bass_utils import run_bass_kernel_spmd

    nc = build_module()
    in_maps = make_in_maps(inputs)
    res = run_bass_kernel_spmd(nc, in_maps, core_ids=list(range(N_CORES)))
    outs = [res.results[i]["out"].reshape(S, CIN, H, W) for i in range(N_CORES)]
    return np.concatenate(outs, axis=0).astype(np.float32)



# revision 2
# speedup vs baseline: 1.0292x; 1.0292x over previous
"""Trainium2 Bass kernel for nn_Bottleneck_5669356834470 (ResNet bottleneck
with an involution middle layer) — v3.

Sharding: data-parallel over batch. 16 samples / 8 cores = 2 samples/core.

Key changes vs v1:
  * involution partition layout (s, g, c-half, m8) -> all 128 partitions used,
    free size per tap op drops 3584 -> 3136 (RH=7 rows per chunk).
  * 10 of the 49 taps run on GpSimd (Pool) concurrently with DVE.
  * PSUM evacuations split across ACT and DVE so neither engine serializes
    the startup (DVE is otherwise idle until the taps begin).
  * out1 DRAM staging is zero-padded per sample so the halo gather is one
    uniform 3-dim DMA per (s,g,h); acc is scattered back channel-major.
  * output written bf16 (host casts to f32) to halve output DMA.
"""

import sys

sys.path.insert(0, "/opt/trn_rl_repo")

import numpy as np
import ml_dtypes

BF16 = ml_dtypes.bfloat16

S = 2            # samples per core
N_CORES = 8
CIN = 256
CMID = 64
G = 4            # involution groups
GC = 16          # channels per group
C8 = 8           # channels per (group, half)
NH = 2           # channel halves per group
KS = 7           # involution kernel size
KK = KS * KS     # 49
R = 16           # dyn-weight bottleneck channels
H = W = 56
HW = H * W       # 3136
RH = 7           # output rows per partition chunk
MCH = 8          # row chunks per (s, g, h)
NP = 128         # partitions = S*G*NH*MCH
NW = RH * W      # 392: matmul / staging chunk (conv1 side)
NWP = RH * 62    # 434: row-padded pixel chunk (w2 / tap side)
NCH = 8          # spatial chunks per sample
HR = 14          # halo rows stored per chunk (13 valid + 1 zero pad)
WP = 62          # padded row width
RUN = 6 * WP + W     # 428: contiguous tap run (7 rows incl inter-row pads)
PAD = 3 * W      # 168: zero margin per sample in out1d
SPX = PAD + HW + PAD   # 3472: out1d pixels per sample
XUF = C8 * HR * W    # 6272 free elems per XU partition
XHF = C8 * HR * WP   # 6944 free elems per XH partition
W2F = KK * NWP       # 21266 free elems per W2T partition (row-padded)
ACCF = C8 * NWP      # 3472 acc free elems per partition (row-padded)
ACCC = C8 * NW       # 3136 compact acc free elems per partition
EPS = 1e-5

# All taps on DVE: concurrent GpSimd tensor ops contend for SBUF and degrade
# DVE throughput ~4x, wiping out any offload gain (measured on HW).
# Taps ordered by k so they can start as soon as the first k-half of the
# weight gather lands.
DVE_TAPS = [(k // KS, k % KS) for k in range(KK)]
KHALF = 25  # w2t gathered in two k-ranges: [0,25) and [25,49)

_CACHE = {}


def _p(s, g, h, m):
    return ((s * G + g) * NH + h) * MCH + m


def _ap(tile_ap, off, dims):
    """Raw strided AP on a tile's underlying tensor. dims=[(step,count),...]
    in elements; for SBUF the partition stride is ap[0][0] of the base AP."""
    import bass_rust

    return bass_rust.AP(tile_ap.tensor, tile_ap.offset + off, [list(d) for d in dims])


def build_module():
    if "nc" in _CACHE:
        return _CACHE["nc"]
    import concourse.bacc as bacc
    import concourse.mybir as mybir
    import concourse.tile as tile

    dt = mybir.dt
    AF = mybir.ActivationFunctionType
    ALU = mybir.AluOpType

    nc = bacc.Bacc("TRN2", debug=False, num_devices=N_CORES)

    # ---- DRAM I/O ----------------------------------------------------------
    xin = nc.dram_tensor("xin", [S, CIN, HW], dt.float32, kind="ExternalInput")
    w1t = nc.dram_tensor("w1t", [2, 128, CMID], dt.bfloat16, kind="ExternalInput")
    b1 = nc.dram_tensor("b1", [CMID, 1], dt.float32, kind="ExternalInput")
    c1t = nc.dram_tensor("c1t", [CMID, R], dt.bfloat16, kind="ExternalInput")
    bi = nc.dram_tensor("bi", [R, 1], dt.float32, kind="ExternalInput")
    c2t = nc.dram_tensor("c2t", [R, G * KK], dt.bfloat16, kind="ExternalInput")
    b2ca = nc.dram_tensor("b2ca", [128, 1], dt.float32, kind="ExternalInput")
    b2cb = nc.dram_tensor("b2cb", [68, 1], dt.float32, kind="ExternalInput")
    s2v = nc.dram_tensor("s2v", [CMID, 1], dt.float32, kind="ExternalInput")
    b2v = nc.dram_tensor("b2v", [CMID, 1], dt.float32, kind="ExternalInput")
    w3t = nc.dram_tensor("w3t", [2, CMID, 128], dt.bfloat16, kind="ExternalInput")
    b3 = nc.dram_tensor("b3", [128, 2], dt.float32, kind="ExternalInput")
    ident = nc.dram_tensor("ident", [128, 128], dt.bfloat16, kind="ExternalInput")
    out = nc.dram_tensor("out", [S, CIN, HW], dt.bfloat16, kind="ExternalOutput")

    with tile.TileContext(nc) as tc:
        with (
            tc.tile_pool(name="consts", bufs=1) as cpool,
            tc.tile_pool(name="big", bufs=1) as bpool,
            tc.tile_pool(name="psum", bufs=8, space="PSUM") as ppool,
            tc.tile_pool(name="zst", bufs=4) as zpool,
            tc.tile_pool(name="rst", bufs=2) as rpool,
            tc.tile_pool(name="ob", bufs=2) as opool,
            tc.tile_pool(name="dstage", bufs=1, space="DRAM") as dpool,
        ):
            # ---- constants -> SBUF ----------------------------------------
            w1t_sb = cpool.tile([128, 2 * CMID], dt.bfloat16, tag="w1t")
            nc.sync.dma_start(
                out=w1t_sb[:, :].rearrange("p (k c) -> p k c", k=2),
                in_=w1t.ap().rearrange("k p c -> p k c"),
            )
            b1_sb = cpool.tile([CMID, 1], dt.float32, tag="b1")
            nc.sync.dma_start(out=b1_sb[:, :], in_=b1.ap())
            c1t_sb = cpool.tile([CMID, R], dt.bfloat16, tag="c1t")
            nc.sync.dma_start(out=c1t_sb[:, :], in_=c1t.ap())
            bi_sb = cpool.tile([R, 1], dt.float32, tag="bi")
            nc.sync.dma_start(out=bi_sb[:, :], in_=bi.ap())
            c2t_sb = cpool.tile([R, G * KK], dt.bfloat16, tag="c2t")
            nc.sync.dma_start(out=c2t_sb[:, :], in_=c2t.ap())
            b2ca_sb = cpool.tile([128, 1], dt.float32, tag="b2ca")
            nc.sync.dma_start(out=b2ca_sb[:, :], in_=b2ca.ap())
            b2cb_sb = cpool.tile([68, 1], dt.float32, tag="b2cb")
            nc.sync.dma_start(out=b2cb_sb[:, :], in_=b2cb.ap())
            s2v_sb = cpool.tile([CMID, 1], dt.float32, tag="s2v")
            nc.sync.dma_start(out=s2v_sb[:, :], in_=s2v.ap())
            b2v_sb = cpool.tile([CMID, 1], dt.float32, tag="b2v")
            nc.sync.dma_start(out=b2v_sb[:, :], in_=b2v.ap())
            w3t_sb = cpool.tile([CMID, 2 * 128], dt.bfloat16, tag="w3t")
            nc.sync.dma_start(
                out=w3t_sb[:, :].rearrange("p (k c) -> p k c", k=2),
                in_=w3t.ap().rearrange("k p c -> p k c"),
            )
            b3_sb = cpool.tile([128, 2], dt.float32, tag="b3")
            nc.sync.dma_start(out=b3_sb[:, :], in_=b3.ap())
            id_sb = cpool.tile([128, 128], dt.bfloat16, tag="ident")
            nc.sync.dma_start(out=id_sb[:, :], in_=ident.ap())

            # ---- big SBUF tiles -------------------------------------------
            xbf = bpool.tile([128, S * 2 * HW], dt.bfloat16, tag="xbf")
            # sized for its out2 reuse (62-padded layout needs S*MCH*NWP=6944)
            out1 = bpool.tile([CMID, S * MCH * NWP], dt.bfloat16, tag="out1")
            w2a = bpool.tile([128, S * MCH * NWP], dt.bfloat16, tag="w2a")
            w2b = bpool.tile([68, S * MCH * NWP], dt.bfloat16, tag="w2b")
            xu = bpool.tile([NP, XUF], dt.bfloat16, tag="xu")
            xh = bpool.tile([NP, XHF], dt.bfloat16, tag="xh")
            xh2 = bpool.tile([NP, XHF], dt.bfloat16, tag="xh2")
            w2t = bpool.tile([NP, W2F], dt.bfloat16, tag="w2t")
            acc_v = bpool.tile([NP, ACCF], dt.bfloat16, tag="acc_v")
            tmp_v = [
                bpool.tile([NP, ACCF], dt.bfloat16, tag=f"tmp_v{i}", name=f"tmp_v{i}")
                for i in range(2)
            ]
            out2 = out1  # out1 is dead after the xu gathers; reuse for out2
            zt = bpool.tile([CMID, PAD], dt.bfloat16, tag="zt")
            zst2 = [
                bpool.tile([R, NWP], dt.bfloat16, tag=f"zst{i}", name=f"zst{i}")
                for i in range(2)
            ]

            xbf_v = xbf[:, :].rearrange("p (s k f) -> p s k f", s=S, k=2)
            xu_v = xu[:, :].rearrange("p (c r w) -> p c r w", c=C8, r=HR, w=W)
            xh_v = xh[:, :].rearrange("p (c r w) -> p c r w", c=C8, r=HR, w=WP)
            xh2_v = xh2[:, :].rearrange("p (c r w) -> p c r w", c=C8, r=HR, w=WP)
            # flat views for the long-run tap ops
            xh_f = xh[:, :].rearrange("p (c f) -> p c f", c=C8)
            xh2_f = xh2[:, :].rearrange("p (c f) -> p c f", c=C8)
            w2t_f = w2t[:, :].rearrange("p (k f) -> p k f", k=KK)
            acc_vf = acc_v[:, :].rearrange("p (c f) -> p c f", c=C8)
            tmp_vf = [t[:, :].rearrange("p (c f) -> p c f", c=C8) for t in tmp_v]
            # strided (row-padded -> compact) view for the final compaction
            acc_vs = acc_v[:, :].rearrange(
                "p (c r w) -> p c r w", c=C8, r=RH, w=WP
            )[:, :, :, 0:W]
            # compact combined acc, reusing tmp_v[0]'s storage
            acc_c = tmp_v[0][:, 0:ACCC]
            acc_cv = acc_c.rearrange("p (c r w) -> p c r w", c=C8, r=RH, w=W)

            # ---- DRAM staging ---------------------------------------------
            # w2d is (s,m)-major so the w2t gather per partition is one
            # contiguous multi-KB run (DRAM-sequential, not 111KB strides).
            out1d = dpool.tile([CMID, S * SPX], dt.bfloat16, tag="out1d")
            w2d = dpool.tile([S * MCH, G * KK * NWP], dt.bfloat16, tag="w2d")
            accd = dpool.tile([CMID, S * MCH * NWP], dt.bfloat16, tag="accd")

            # ---- memzeros (pads for halo tensors) -------------------------
            nc.scalar.memzero(xu[:, :])
            nc.scalar.memzero(xh[:, :])
            nc.vector.memset(xh2[:, :], 0.0)
            nc.vector.memset(zt[:, :], 0.0)
            nc.vector.memset(acc_v[:, :], 0.0)
            for z in zst2:
                nc.vector.memset(z[:, :], 0.0)

            # zero margins of out1d so halo gathers read zeros off the edges
            for s in range(S):
                nc.sync.dma_start(
                    out=out1d[:, s * SPX : s * SPX + PAD], in_=zt[:, :]
                )
                nc.sync.dma_start(
                    out=out1d[:, s * SPX + PAD + HW : (s + 1) * SPX], in_=zt[:, :]
                )

            # ---- x load (f32 -> bf16 cast; SWDGE on gpsimd) ---------------
            for s in range(S):
                for kc in range(2):
                    nc.gpsimd.dma_start(
                        out=xbf_v[:, s, kc, :],
                        in_=xin.ap()[s, kc * 128 : (kc + 1) * 128, :],
                    )

            # ---- per-sample front end: conv1 / inv_c1 / inv_c2 ------------
            w1t_v = w1t_sb[:, :].rearrange("p (k c) -> p k c", k=2)
            o1d_ap = out1d[:, :]
            w2d_ap = w2d[:, :]
            xu_ap = xu[:, :]
            w2t_ap = w2t[:, :]
            D1 = o1d_ap.ap[0][0]
            D2 = w2d_ap.ap[0][0]
            P_XU = xu_ap.ap[0][0]
            P_W2T = w2t_ap.ap[0][0]

            for s in range(S):
                for n in range(NCH):
                    sl = slice(s * HW + n * NW, s * HW + (n + 1) * NW)
                    # conv1 (256->64) + BN1 + ReLU   [ACT evac]
                    ps = ppool.tile([128, NW], dt.float32, tag="ps", bufs=4)
                    for kc in range(2):
                        nc.tensor.matmul(
                            ps[:CMID, :],
                            w1t_v[:, kc, :],
                            xbf_v[:, s, kc, n * NW : (n + 1) * NW],
                            start=(kc == 0),
                            stop=(kc == 1),
                        )
                    nc.scalar.activation(
                        out1[:, sl], ps[:CMID, :], AF.Relu, bias=b1_sb[:, 0:1]
                    )
                    # inv_c1 (64->16) + BN + ReLU    [DVE evac]
                    # zst is row-padded to 62-wide rows; the pad columns keep
                    # stale (finite) values which flow through inv_c2 into pad
                    # weight columns that only ever multiply zeros.
                    ps1 = ppool.tile([128, NW], dt.float32, tag="ps", bufs=4)
                    nc.tensor.matmul(
                        ps1[:R, :], c1t_sb[:, :], out1[:, sl], start=True, stop=True
                    )
                    zst = zst2[n % 2]
                    zst_v = zst[:, :].rearrange("p (r w) -> p r w", r=RH, w=WP)
                    nc.vector.tensor_scalar(
                        zst_v[:, :, 0:W],
                        ps1[:R, :].rearrange("p (r w) -> p r w", r=RH, w=W),
                        bi_sb[:, 0:1],
                        0.0,
                        op0=ALU.add,
                        op1=ALU.max,
                    )
                    # inv_c2 (16->196) + bias        [DVE + ACT evacs]
                    psa = ppool.tile([128, NWP], dt.float32, tag="psw", bufs=4)
                    psb = ppool.tile([128, NWP], dt.float32, tag="psw", bufs=4)
                    nc.tensor.matmul(
                        psa[:, :], c2t_sb[:, 0:128], zst[:, :], start=True, stop=True
                    )
                    nc.tensor.matmul(
                        psb[:68, :], c2t_sb[:, 128:196], zst[:, :], start=True, stop=True
                    )
                    nsl = slice((s * MCH + n) * NWP, (s * MCH + n + 1) * NWP)
                    nc.vector.tensor_scalar(
                        w2a[:, nsl], psa[:, :], b2ca_sb[:, 0:1], None, op0=ALU.add
                    )
                    nc.scalar.activation(
                        w2b[:, nsl], psb[:68, :], AF.Identity, bias=b2cb_sb[:, 0:1]
                    )

                # stage out1 through DRAM ------------------------------------
                nc.sync.dma_start(
                    out=out1d[:, s * SPX + PAD : s * SPX + PAD + HW],
                    in_=out1[:, s * HW : (s + 1) * HW],
                )

                # xu gathers on the SWDGE queue (16-engine, parallel with SP);
                # c-outer/m-inner so source reads walk DRAM monotonically
                for g in range(G):
                    for h in range(NH):
                        nc.gpsimd.dma_start(
                            out=_ap(
                                xu_ap,
                                _p(s, g, h, 0) * P_XU,
                                [(P_XU, MCH), (HR * W, C8), (1, 13 * W)],
                            ),
                            in_=_ap(
                                o1d_ap,
                                (g * GC + h * C8) * D1 + s * SPX,
                                [(NW, MCH), (D1, C8), (1, 13 * W)],
                            ),
                        )

            # w2 -> (s,m)-major DRAM after both samples (keeps the SP queue
            # free for out1d during the startup-critical window); dst runs
            # are gk-sequential 868B
            nc.sync.dma_start(
                out=_ap(w2d_ap, 0, [(NWP, 128), (D2, S * MCH), (1, NWP)]),
                in_=_ap(
                    w2a[:, :],
                    0,
                    [(w2a[:, :].ap[0][0], 128), (NWP, S * MCH), (1, NWP)],
                ),
            )
            nc.sync.dma_start(
                out=_ap(w2d_ap, 128 * NWP, [(NWP, 68), (D2, S * MCH), (1, NWP)]),
                in_=_ap(
                    w2b[:, :],
                    0,
                    [(w2b[:, :].ap[0][0], 68), (NWP, S * MCH), (1, NWP)],
                ),
            )

            # w2t gathers: one contiguous run per (partition, k-half); all
            # first-half gathers first so low-k taps can start early
            for kh0, kn in ((0, KHALF), (KHALF, KK - KHALF)):
                for s in range(S):
                    for g in range(G):
                        for h in range(NH):
                            nc.sync.dma_start(
                                out=_ap(
                                    w2t_ap,
                                    _p(s, g, h, 0) * P_W2T + kh0 * NWP,
                                    [(P_W2T, MCH), (1, kn * NWP)],
                                ),
                                in_=_ap(
                                    w2d_ap,
                                    s * MCH * D2 + (g * KK + kh0) * NWP,
                                    [(D2, MCH), (1, kn * NWP)],
                                ),
                            )

            # ---- halo expansion: XU -> XH (ACT), XH2 (DVE) ----------------
            nc.scalar.copy(xh_v[:, :, :, 3 : 3 + W], xu_v)
            nc.vector.tensor_copy(xh2_v[:, :, :, 4 : 4 + W], xu_v)

            # ---- involution taps: DVE + GpSimd ----------------------------
            # Long-run form: each tap is one contiguous 428-elem run per
            # (partition, c). The 6 inter-row pad positions read zeros from
            # xh/xh2 pads, so the junk they deposit in acc's pad columns is
            # exactly 0 and gets dropped by the compacting combine.
            def tap_ops(eng, taps, accf, tmpf):
                for i, (kh, kw) in enumerate(taps):
                    if kw % 2 == 0:
                        base = kh * WP + kw
                        in0 = xh_f[:, :, base : base + RUN]
                    else:
                        base = kh * WP + kw + 1
                        in0 = xh2_f[:, :, base : base + RUN]
                    k = kh * KS + kw
                    in1 = w2t_f[:, k : k + 1, 0:RUN].to_broadcast([NP, C8, RUN])
                    if i == 0:
                        eng.tensor_mul(accf[:, :, 0:RUN], in0, in1)
                    else:
                        t = tmpf[i % len(tmpf)]
                        eng.tensor_mul(t[:, :, 0:RUN], in0, in1)
                        eng.tensor_add(
                            accf[:, :, 0:RUN], accf[:, :, 0:RUN], t[:, :, 0:RUN]
                        )

            tap_ops(nc.vector, DVE_TAPS, acc_vf, tmp_vf)

            # ---- acc -> DRAM channel-major scatter (62-padded throughout;
            # the pad columns carry exact zeros / ignorable junk) ------------
            acd_ap = accd[:, :]
            D3 = acd_ap.ap[0][0]
            acc_ap = acc_v[:, :]
            P_AC = acc_ap.ap[0][0]
            # s0 half on the SWDGE queue, s1 half on SP: parallel desc-gen
            for s in range(S):
                eng = nc.gpsimd if s == 0 else nc.sync
                for g in range(G):
                    for h in range(NH):
                        eng.dma_start(
                            out=_ap(
                                acd_ap,
                                (g * GC + h * C8) * D3 + s * MCH * NWP,
                                [(NWP, MCH), (D3, C8), (1, NWP)],
                            ),
                            in_=_ap(
                                acc_ap,
                                _p(s, g, h, 0) * P_AC,
                                [(P_AC, MCH), (NWP, C8), (1, NWP)],
                            ),
                        )
            # out2 <- accd (channel-major now; plain reads). out2 reuses the
            # (dead) out1 tile's storage, in the 62-padded layout.
            SMW = MCH * NWP  # 3472 padded pixels per sample
            out2v = out1[:, 0 : S * SMW]
            for s in range(S):
                nc.sync.dma_start(
                    out=out2v[:, s * SMW : (s + 1) * SMW],
                    in_=accd[:, s * SMW : (s + 1) * SMW],
                )

            # ---- BN2 + ReLU; conv3 + BN3 + residual + ReLU ----------------
            w3t_v = w3t_sb[:, :].rearrange("p (k c) -> p k c", k=2)
            eng_flip = 0
            for s in range(S):
                obufs = [
                    opool.tile([128, HW], dt.bfloat16, tag="ob", name=f"ob{s}_{i}")
                    for i in range(2)
                ]
                for q in range(4):
                    rst = rpool.tile([CMID, 2 * NWP], dt.bfloat16, tag="rst")
                    nc.scalar.activation(
                        rst[:, :],
                        out2v[:, s * SMW + q * 2 * NWP : s * SMW + (q + 1) * 2 * NWP],
                        AF.Relu,
                        bias=b2v_sb[:, 0:1],
                        scale=s2v_sb[:, 0:1],
                    )
                    rst_v = rst[:, :].rearrange(
                        "p (m r w) -> p m r w", m=2, r=RH, w=WP
                    )
                    for oc in range(2):
                        for hf in range(2):
                            ps = ppool.tile([128, NW], dt.float32, tag="ps", bufs=4)
                            nc.tensor.matmul(
                                ps[:, :],
                                w3t_v[:, oc, :],
                                rst_v[:, hf, :, 0:W],
                                start=True,
                                stop=False,
                            )
                            nx = (q * 2 + hf) * NW
                            nc.tensor.matmul(
                                ps[:, :],
                                id_sb[:, :],
                                xbf_v[:, s, oc, nx : nx + NW],
                                start=False,
                                stop=True,
                            )
                            dst = obufs[oc][:, nx : nx + NW]
                            if eng_flip % 2 == 0:
                                nc.vector.tensor_scalar(
                                    dst,
                                    ps[:, :],
                                    b3_sb[:, oc : oc + 1],
                                    0.0,
                                    op0=ALU.add,
                                    op1=ALU.max,
                                )
                            else:
                                nc.scalar.activation(
                                    dst, ps[:, :], AF.Relu, bias=b3_sb[:, oc : oc + 1]
                                )
                            eng_flip += 1
                for oc in range(2):
                    nc.sync.dma_start(
                        out=out.ap()[s, oc * 128 : (oc + 1) * 128, :],
                        in_=obufs[oc][:, :],
                    )

    nc.compile()
    _CACHE["nc"] = nc
    return nc


def _f32(a):
    return np.ascontiguousarray(a, dtype=np.float32)


def prep_weights(inputs):
    """Host-side folding of BN scales into conv weights; bf16 casts."""
    f = inputs
    s1 = f["bn1_g"] / np.sqrt(f["bn1_v"] + EPS)
    b1_eff = f["bn1_b"] - f["bn1_m"] * s1
    w1t_eff = (_f32(f["conv1_w"]) * s1[:, None]).T          # [256, 64]

    si = f["inv_bn_g"] / np.sqrt(f["inv_bn_v"] + EPS)
    bi_eff = f["inv_bn_b"] - f["inv_bn_m"] * si
    c1t_eff = (_f32(f["inv_c1_w"]) * si[:, None]).T         # [64, 16]

    c2t_eff = _f32(f["inv_c2_w"]).T                         # [16, 196]
    b2c = _f32(f["inv_c2_b"])

    s2 = f["bn2_g"] / np.sqrt(f["bn2_v"] + EPS)
    b2n = f["bn2_b"] - f["bn2_m"] * s2

    s3 = f["bn3_g"] / np.sqrt(f["bn3_v"] + EPS)
    b3_eff = f["bn3_b"] - f["bn3_m"] * s3
    w3t_eff = (_f32(f["conv3_w"]) * s3[:, None]).T          # [64, 256]

    d = {}
    d["w1t"] = np.ascontiguousarray(w1t_eff.reshape(2, 128, CMID).astype(BF16))
    d["b1"] = _f32(b1_eff)[:, None]
    d["c1t"] = np.ascontiguousarray(c1t_eff.astype(BF16))
    d["bi"] = _f32(bi_eff)[:, None]
    d["c2t"] = np.ascontiguousarray(c2t_eff.astype(BF16))
    d["b2ca"] = _f32(b2c[0:128])[:, None]
    d["b2cb"] = _f32(b2c[128:196])[:, None]
    d["s2v"] = _f32(s2)[:, None]
    d["b2v"] = _f32(b2n)[:, None]
    d["w3t"] = np.ascontiguousarray(
        w3t_eff.reshape(CMID, 2, 128).transpose(1, 0, 2).astype(BF16)
    )
    d["b3"] = _f32(b3_eff.reshape(2, 128).T)
    d["ident"] = np.ascontiguousarray(np.eye(128, dtype=np.float32).astype(BF16))
    return d


def make_in_maps(inputs):
    prep = prep_weights(inputs)
    x = _f32(inputs["x"]).reshape(16, CIN, HW)
    in_maps = []
    for i in range(N_CORES):
        m = dict(prep)
        m["xin"] = np.ascontiguousarray(x[S * i : S * i + S])
        in_maps.append(m)
    return in_maps


def kernel(**inputs):
    from concourse.bass_utils import run_bass_kernel_spmd

    nc = build_module()
    in_maps = make_in_maps(inputs)
    res = run_bass_kernel_spmd(nc, in_maps, core_ids=list(range(N_CORES)))
    outs = [
        np.asarray(res.results[i]["out"], dtype=np.float32).reshape(S, CIN, H, W)
        for i in range(N_CORES)
    ]
    return np.concatenate(outs, axis=0).astype(np.float32)


# revision 3
# speedup vs baseline: 1.0425x; 1.0129x over previous
"""Trainium2 Bass kernel for nn_Bottleneck_5669356834470 (ResNet bottleneck
with an involution middle layer) — v3.

Sharding: data-parallel over batch. 16 samples / 8 cores = 2 samples/core.

Key changes vs v1:
  * involution partition layout (s, g, c-half, m8) -> all 128 partitions used,
    free size per tap op drops 3584 -> 3136 (RH=7 rows per chunk).
  * 10 of the 49 taps run on GpSimd (Pool) concurrently with DVE.
  * PSUM evacuations split across ACT and DVE so neither engine serializes
    the startup (DVE is otherwise idle until the taps begin).
  * out1 DRAM staging is zero-padded per sample so the halo gather is one
    uniform 3-dim DMA per (s,g,h); acc is scattered back channel-major.
  * output written bf16 (host casts to f32) to halve output DMA.
"""

import sys

sys.path.insert(0, "/opt/trn_rl_repo")

import numpy as np
import ml_dtypes

BF16 = ml_dtypes.bfloat16

S = 2            # samples per core
N_CORES = 8
CIN = 256
CMID = 64
G = 4            # involution groups
GC = 16          # channels per group
C8 = 8           # channels per (group, half)
NH = 2           # channel halves per group
KS = 7           # involution kernel size
KK = KS * KS     # 49
R = 16           # dyn-weight bottleneck channels
H = W = 56
HW = H * W       # 3136
RH = 7           # output rows per partition chunk
MCH = 8          # row chunks per (s, g, h)
NP = 128         # partitions = S*G*NH*MCH
NW = RH * W      # 392: matmul / staging chunk (conv1 side)
NWP = RH * 62    # 434: row-padded pixel chunk (w2 / tap side)
NCH = 8          # spatial chunks per sample
HR = 14          # halo rows stored per chunk (13 valid + 1 zero pad)
WP = 62          # padded row width
RUN = 6 * WP + W     # 428: contiguous tap run (7 rows incl inter-row pads)
PAD = 3 * W      # 168: zero margin per sample in out1d
SPX = PAD + HW + PAD   # 3472: out1d pixels per sample
XUF = C8 * HR * W    # 6272 free elems per XU partition
XHF = C8 * HR * WP   # 6944 free elems per XH partition
W2F = KK * NWP       # 21266 free elems per W2T partition (row-padded)
ACCF = C8 * NWP      # 3472 acc free elems per partition (row-padded)
ACCC = C8 * NW       # 3136 compact acc free elems per partition
EPS = 1e-5

# All taps on DVE: concurrent GpSimd tensor ops contend for SBUF and degrade
# DVE throughput ~4x, wiping out any offload gain (measured on HW).
# Taps ordered by k so they can start as soon as the first k-half of the
# weight gather lands.
DVE_TAPS = [(k // KS, k % KS) for k in range(KK)]
KHALF = 25  # w2t gathered in two k-ranges: [0,25) and [25,49)

_CACHE = {}


def _p(s, g, h, m):
    return ((s * G + g) * NH + h) * MCH + m


def _ap(tile_ap, off, dims):
    """Raw strided AP on a tile's underlying tensor. dims=[(step,count),...]
    in elements; for SBUF the partition stride is ap[0][0] of the base AP."""
    import bass_rust

    return bass_rust.AP(tile_ap.tensor, tile_ap.offset + off, [list(d) for d in dims])


def build_module():
    if "nc" in _CACHE:
        return _CACHE["nc"]
    import concourse.bacc as bacc
    import concourse.mybir as mybir
    import concourse.tile as tile

    dt = mybir.dt
    AF = mybir.ActivationFunctionType
    ALU = mybir.AluOpType

    nc = bacc.Bacc("TRN2", debug=False, num_devices=N_CORES)

    # ---- DRAM I/O ----------------------------------------------------------
    xin = nc.dram_tensor("xin", [S, CIN, HW], dt.float32, kind="ExternalInput")
    w1t = nc.dram_tensor("w1t", [2, 128, CMID], dt.bfloat16, kind="ExternalInput")
    b1 = nc.dram_tensor("b1", [CMID, 1], dt.float32, kind="ExternalInput")
    c1t = nc.dram_tensor("c1t", [CMID, R], dt.bfloat16, kind="ExternalInput")
    bi = nc.dram_tensor("bi", [R, 1], dt.float32, kind="ExternalInput")
    c2t = nc.dram_tensor("c2t", [R, G * KK], dt.bfloat16, kind="ExternalInput")
    b2ca = nc.dram_tensor("b2ca", [128, 1], dt.float32, kind="ExternalInput")
    b2cb = nc.dram_tensor("b2cb", [68, 1], dt.float32, kind="ExternalInput")
    s2v = nc.dram_tensor("s2v", [CMID, 1], dt.float32, kind="ExternalInput")
    b2v = nc.dram_tensor("b2v", [CMID, 1], dt.float32, kind="ExternalInput")
    w3t = nc.dram_tensor("w3t", [2, CMID, 128], dt.bfloat16, kind="ExternalInput")
    b3 = nc.dram_tensor("b3", [128, 2], dt.float32, kind="ExternalInput")
    ident = nc.dram_tensor("ident", [128, 128], dt.bfloat16, kind="ExternalInput")
    out = nc.dram_tensor("out", [S, CIN, HW], dt.bfloat16, kind="ExternalOutput")

    with tile.TileContext(nc) as tc:
        with (
            tc.tile_pool(name="consts", bufs=1) as cpool,
            tc.tile_pool(name="big", bufs=1) as bpool,
            tc.tile_pool(name="psum", bufs=8, space="PSUM") as ppool,
            tc.tile_pool(name="zst", bufs=4) as zpool,
            tc.tile_pool(name="rst", bufs=2) as rpool,
            tc.tile_pool(name="ob", bufs=2) as opool,
            tc.tile_pool(name="dstage", bufs=1, space="DRAM") as dpool,
        ):
            # ---- constants -> SBUF ----------------------------------------
            w1t_sb = cpool.tile([128, 2 * CMID], dt.bfloat16, tag="w1t")
            nc.sync.dma_start(
                out=w1t_sb[:, :].rearrange("p (k c) -> p k c", k=2),
                in_=w1t.ap().rearrange("k p c -> p k c"),
            )
            b1_sb = cpool.tile([CMID, 1], dt.float32, tag="b1")
            nc.sync.dma_start(out=b1_sb[:, :], in_=b1.ap())
            c1t_sb = cpool.tile([CMID, R], dt.bfloat16, tag="c1t")
            nc.sync.dma_start(out=c1t_sb[:, :], in_=c1t.ap())
            bi_sb = cpool.tile([R, 1], dt.float32, tag="bi")
            nc.sync.dma_start(out=bi_sb[:, :], in_=bi.ap())
            c2t_sb = cpool.tile([R, G * KK], dt.bfloat16, tag="c2t")
            nc.sync.dma_start(out=c2t_sb[:, :], in_=c2t.ap())
            b2ca_sb = cpool.tile([128, 1], dt.float32, tag="b2ca")
            nc.sync.dma_start(out=b2ca_sb[:, :], in_=b2ca.ap())
            b2cb_sb = cpool.tile([68, 1], dt.float32, tag="b2cb")
            nc.sync.dma_start(out=b2cb_sb[:, :], in_=b2cb.ap())
            s2v_sb = cpool.tile([CMID, 1], dt.float32, tag="s2v")
            nc.sync.dma_start(out=s2v_sb[:, :], in_=s2v.ap())
            b2v_sb = cpool.tile([CMID, 1], dt.float32, tag="b2v")
            nc.sync.dma_start(out=b2v_sb[:, :], in_=b2v.ap())
            w3t_sb = cpool.tile([CMID, 2 * 128], dt.bfloat16, tag="w3t")
            nc.sync.dma_start(
                out=w3t_sb[:, :].rearrange("p (k c) -> p k c", k=2),
                in_=w3t.ap().rearrange("k p c -> p k c"),
            )
            b3_sb = cpool.tile([128, 2], dt.float32, tag="b3")
            nc.sync.dma_start(out=b3_sb[:, :], in_=b3.ap())
            id_sb = cpool.tile([128, 128], dt.bfloat16, tag="ident")
            nc.sync.dma_start(out=id_sb[:, :], in_=ident.ap())

            # ---- big SBUF tiles -------------------------------------------
            xbf = bpool.tile([128, S * 2 * HW], dt.bfloat16, tag="xbf")
            # sized for its out2 reuse (62-padded layout needs S*MCH*NWP=6944)
            out1 = bpool.tile([CMID, S * MCH * NWP], dt.bfloat16, tag="out1")
            w2a = bpool.tile([128, S * MCH * NWP], dt.bfloat16, tag="w2a")
            w2b = bpool.tile([68, S * MCH * NWP], dt.bfloat16, tag="w2b")
            xu = bpool.tile([NP, XUF], dt.bfloat16, tag="xu")
            xh = bpool.tile([NP, XHF], dt.bfloat16, tag="xh")
            xh2 = bpool.tile([NP, XHF], dt.bfloat16, tag="xh2")
            w2t = bpool.tile([NP, W2F], dt.bfloat16, tag="w2t")
            acc2 = bpool.tile([NP, 2 * ACCF], dt.bfloat16, tag="acc2")
            tmp2 = [
                bpool.tile([NP, 2 * ACCF], dt.bfloat16, tag=f"tmp2_{i}", name=f"tmp2_{i}")
                for i in range(2)
            ]
            out2 = out1  # out1 is dead after the xu gathers; reuse for out2
            zt = bpool.tile([CMID, PAD], dt.bfloat16, tag="zt")
            zst2 = [
                bpool.tile([R, NWP], dt.bfloat16, tag=f"zst{i}", name=f"zst{i}")
                for i in range(2)
            ]

            xbf_v = xbf[:, :].rearrange("p (s k f) -> p s k f", s=S, k=2)
            xu_v = xu[:, :].rearrange("p (c r w) -> p c r w", c=C8, r=HR, w=W)
            xh_v = xh[:, :].rearrange("p (c r w) -> p c r w", c=C8, r=HR, w=WP)
            xh2_v = xh2[:, :].rearrange("p (c r w) -> p c r w", c=C8, r=HR, w=WP)
            # flat views for the long-run tap ops
            xh_f = xh[:, :].rearrange("p (c f) -> p c f", c=C8)
            xh2_f = xh2[:, :].rearrange("p (c f) -> p c f", c=C8)
            P_XH = xh[:, :].ap[0][0]
            P_XH2 = xh2[:, :].ap[0][0]
            P_A2 = acc2[:, :].ap[0][0]
            P_T2 = [t[:, :].ap[0][0] for t in tmp2]

            # ---- DRAM staging ---------------------------------------------
            # w2d is (s,m)-major so the w2t gather per partition is one
            # contiguous multi-KB run (DRAM-sequential, not 111KB strides).
            out1d = dpool.tile([CMID, S * SPX], dt.bfloat16, tag="out1d")
            w2d = dpool.tile([S * MCH, G * KK * NWP], dt.bfloat16, tag="w2d")
            accd = dpool.tile([CMID, S * MCH * NWP], dt.bfloat16, tag="accd")

            # ---- memzeros (pads for halo tensors) -------------------------
            nc.scalar.memzero(xu[:, :])
            nc.scalar.memzero(xh[:, :])
            nc.vector.memset(xh2[:, :], 0.0)
            nc.vector.memset(zt[:, :], 0.0)
            nc.vector.memset(acc2[:, :], 0.0)
            for z in zst2:
                nc.vector.memset(z[:, :], 0.0)

            # zero margins of out1d so halo gathers read zeros off the edges
            for s in range(S):
                nc.sync.dma_start(
                    out=out1d[:, s * SPX : s * SPX + PAD], in_=zt[:, :]
                )
                nc.sync.dma_start(
                    out=out1d[:, s * SPX + PAD + HW : (s + 1) * SPX], in_=zt[:, :]
                )

            # ---- x load (f32 -> bf16 cast; SWDGE on gpsimd) ---------------
            for s in range(S):
                for kc in range(2):
                    nc.gpsimd.dma_start(
                        out=xbf_v[:, s, kc, :],
                        in_=xin.ap()[s, kc * 128 : (kc + 1) * 128, :],
                    )

            # ---- per-sample front end: conv1 / inv_c1 / inv_c2 ------------
            w1t_v = w1t_sb[:, :].rearrange("p (k c) -> p k c", k=2)
            o1d_ap = out1d[:, :]
            w2d_ap = w2d[:, :]
            xu_ap = xu[:, :]
            w2t_ap = w2t[:, :]
            D1 = o1d_ap.ap[0][0]
            D2 = w2d_ap.ap[0][0]
            P_XU = xu_ap.ap[0][0]
            P_W2T = w2t_ap.ap[0][0]

            for s in range(S):
                for n in range(NCH):
                    sl = slice(s * HW + n * NW, s * HW + (n + 1) * NW)
                    # conv1 (256->64) + BN1 + ReLU   [ACT evac]
                    ps = ppool.tile([128, NW], dt.float32, tag="ps", bufs=4)
                    for kc in range(2):
                        nc.tensor.matmul(
                            ps[:CMID, :],
                            w1t_v[:, kc, :],
                            xbf_v[:, s, kc, n * NW : (n + 1) * NW],
                            start=(kc == 0),
                            stop=(kc == 1),
                        )
                    nc.scalar.activation(
                        out1[:, sl], ps[:CMID, :], AF.Relu, bias=b1_sb[:, 0:1]
                    )
                    # inv_c1 (64->16) + BN + ReLU    [DVE evac]
                    # zst is row-padded to 62-wide rows; the pad columns keep
                    # stale (finite) values which flow through inv_c2 into pad
                    # weight columns that only ever multiply zeros.
                    ps1 = ppool.tile([128, NW], dt.float32, tag="ps", bufs=4)
                    nc.tensor.matmul(
                        ps1[:R, :], c1t_sb[:, :], out1[:, sl], start=True, stop=True
                    )
                    zst = zst2[n % 2]
                    zst_v = zst[:, :].rearrange("p (r w) -> p r w", r=RH, w=WP)
                    nc.vector.tensor_scalar(
                        zst_v[:, :, 0:W],
                        ps1[:R, :].rearrange("p (r w) -> p r w", r=RH, w=W),
                        bi_sb[:, 0:1],
                        0.0,
                        op0=ALU.add,
                        op1=ALU.max,
                    )
                    # inv_c2 (16->196) + bias        [DVE + ACT evacs]
                    psa = ppool.tile([128, NWP], dt.float32, tag="psw", bufs=4)
                    psb = ppool.tile([128, NWP], dt.float32, tag="psw", bufs=4)
                    nc.tensor.matmul(
                        psa[:, :], c2t_sb[:, 0:128], zst[:, :], start=True, stop=True
                    )
                    nc.tensor.matmul(
                        psb[:68, :], c2t_sb[:, 128:196], zst[:, :], start=True, stop=True
                    )
                    nsl = slice((s * MCH + n) * NWP, (s * MCH + n + 1) * NWP)
                    nc.vector.tensor_scalar(
                        w2a[:, nsl], psa[:, :], b2ca_sb[:, 0:1], None, op0=ALU.add
                    )
                    nc.scalar.activation(
                        w2b[:, nsl], psb[:68, :], AF.Identity, bias=b2cb_sb[:, 0:1]
                    )

                # stage out1 through DRAM ------------------------------------
                nc.sync.dma_start(
                    out=out1d[:, s * SPX + PAD : s * SPX + PAD + HW],
                    in_=out1[:, s * HW : (s + 1) * HW],
                )

                # xu gathers on the SWDGE queue (16-engine, parallel with SP);
                # c-outer/m-inner so source reads walk DRAM monotonically
                for g in range(G):
                    for h in range(NH):
                        nc.gpsimd.dma_start(
                            out=_ap(
                                xu_ap,
                                _p(s, g, h, 0) * P_XU,
                                [(P_XU, MCH), (HR * W, C8), (1, 13 * W)],
                            ),
                            in_=_ap(
                                o1d_ap,
                                (g * GC + h * C8) * D1 + s * SPX,
                                [(NW, MCH), (D1, C8), (1, 13 * W)],
                            ),
                        )

            # w2 -> (s,m)-major DRAM after both samples (keeps the SP queue
            # free for out1d during the startup-critical window); dst runs
            # are gk-sequential 868B
            nc.sync.dma_start(
                out=_ap(w2d_ap, 0, [(NWP, 128), (D2, S * MCH), (1, NWP)]),
                in_=_ap(
                    w2a[:, :],
                    0,
                    [(w2a[:, :].ap[0][0], 128), (NWP, S * MCH), (1, NWP)],
                ),
            )
            nc.sync.dma_start(
                out=_ap(w2d_ap, 128 * NWP, [(NWP, 68), (D2, S * MCH), (1, NWP)]),
                in_=_ap(
                    w2b[:, :],
                    0,
                    [(w2b[:, :].ap[0][0], 68), (NWP, S * MCH), (1, NWP)],
                ),
            )

            # w2t gathers: one contiguous run per (partition, k-range);
            # ranges match the kh-pair groups so the first pair-group's taps
            # can start after only the first quarter lands
            for kh0, kn in ((0, 14), (14, 14), (28, 14), (42, 7)):
                for s in range(S):
                    for g in range(G):
                        for h in range(NH):
                            nc.sync.dma_start(
                                out=_ap(
                                    w2t_ap,
                                    _p(s, g, h, 0) * P_W2T + kh0 * NWP,
                                    [(P_W2T, MCH), (1, kn * NWP)],
                                ),
                                in_=_ap(
                                    w2d_ap,
                                    s * MCH * D2 + (g * KK + kh0) * NWP,
                                    [(D2, MCH), (1, kn * NWP)],
                                ),
                            )

            # ---- halo expansion: XU -> XH (ACT), XH2 (DVE) ----------------
            nc.scalar.copy(xh_v[:, :, :, 3 : 3 + W], xu_v)
            nc.vector.tensor_copy(xh2_v[:, :, :, 4 : 4 + W], xu_v)

            # ---- involution taps: kh-pairs merged into single DVE ops ----
            # Pair (kh, kh+1) for kh in {0,2,4} plus single kh=6, per kw.
            # Each pair op has a k2 dim (stride 62 in xh, 7*NWP in w2t,
            # ACCF in acc2) -> two partial sums in acc2's slots; one final
            # slot-add after all taps. ~41 fewer DVE op overheads.
            w2t_raw = w2t[:, :]
            first = True
            ti = 0
            for kh in (0, 2, 4, 6):
                npair = 2 if kh < 6 else 1
                for kw in range(KS):
                    if kw % 2 == 0:
                        srcap, psrc, base = xh[:, :], P_XH, kh * WP + kw
                    else:
                        srcap, psrc, base = xh2[:, :], P_XH2, kh * WP + kw + 1
                    k = kh * KS + kw
                    in0 = _ap(
                        srcap, base,
                        [(psrc, NP), (WP, npair), (868, C8), (1, RUN)],
                    )
                    in1 = _ap(
                        w2t_raw, k * NWP,
                        [(P_W2T, NP), (KS * NWP, npair), (0, C8), (1, RUN)],
                    )
                    if first:
                        dst = _ap(
                            acc2[:, :], 0,
                            [(P_A2, NP), (ACCF, npair), (NWP, C8), (1, RUN)],
                        )
                        nc.vector.tensor_mul(dst, in0, in1)
                        first = False
                    else:
                        t = ti % 2
                        tdst = _ap(
                            tmp2[t][:, :], 0,
                            [(P_T2[t], NP), (ACCF, npair), (NWP, C8), (1, RUN)],
                        )
                        adst = _ap(
                            acc2[:, :], 0,
                            [(P_A2, NP), (ACCF, npair), (NWP, C8), (1, RUN)],
                        )
                        nc.vector.tensor_mul(tdst, in0, in1)
                        nc.vector.tensor_add(adst, adst, tdst)
                        ti += 1
            # fold slot1 into slot0
            s0 = _ap(acc2[:, :], 0, [(P_A2, NP), (1, ACCF)])
            s1 = _ap(acc2[:, :], ACCF, [(P_A2, NP), (1, ACCF)])
            nc.vector.tensor_add(s0, s0, s1)

            # ---- acc -> DRAM channel-major scatter (62-padded throughout;
            # the pad columns carry exact zeros / ignorable junk) ------------
            acd_ap = accd[:, :]
            D3 = acd_ap.ap[0][0]
            acc_ap = acc2[:, :]
            P_AC = P_A2
            # s0 half on the SWDGE queue, s1 half on SP: parallel desc-gen
            for s in range(S):
                eng = nc.gpsimd if s == 0 else nc.sync
                for g in range(G):
                    for h in range(NH):
                        eng.dma_start(
                            out=_ap(
                                acd_ap,
                                (g * GC + h * C8) * D3 + s * MCH * NWP,
                                [(NWP, MCH), (D3, C8), (1, NWP)],
                            ),
                            in_=_ap(
                                acc_ap,
                                _p(s, g, h, 0) * P_AC,
                                [(P_AC, MCH), (NWP, C8), (1, NWP)],
                            ),
                        )
            # out2 <- accd (channel-major now; plain reads). out2 reuses the
            # (dead) out1 tile's storage, in the 62-padded layout.
            SMW = MCH * NWP  # 3472 padded pixels per sample
            out2v = out1[:, 0 : S * SMW]
            for s in range(S):
                nc.sync.dma_start(
                    out=out2v[:, s * SMW : (s + 1) * SMW],
                    in_=accd[:, s * SMW : (s + 1) * SMW],
                )

            # ---- BN2 + ReLU; conv3 + BN3 + residual + ReLU ----------------
            w3t_v = w3t_sb[:, :].rearrange("p (k c) -> p k c", k=2)
            eng_flip = 0
            for s in range(S):
                obufs = [
                    opool.tile([128, HW], dt.bfloat16, tag="ob", name=f"ob{s}_{i}")
                    for i in range(2)
                ]
                for q in range(4):
                    rst = rpool.tile([CMID, 2 * NWP], dt.bfloat16, tag="rst")
                    nc.scalar.activation(
                        rst[:, :],
                        out2v[:, s * SMW + q * 2 * NWP : s * SMW + (q + 1) * 2 * NWP],
                        AF.Relu,
                        bias=b2v_sb[:, 0:1],
                        scale=s2v_sb[:, 0:1],
                    )
                    rst_v = rst[:, :].rearrange(
                        "p (m r w) -> p m r w", m=2, r=RH, w=WP
                    )
                    for oc in range(2):
                        for hf in range(2):
                            ps = ppool.tile([128, NW], dt.float32, tag="ps", bufs=4)
                            nc.tensor.matmul(
                                ps[:, :],
                                w3t_v[:, oc, :],
                                rst_v[:, hf, :, 0:W],
                                start=True,
                                stop=False,
                            )
                            nx = (q * 2 + hf) * NW
                            nc.tensor.matmul(
                                ps[:, :],
                                id_sb[:, :],
                                xbf_v[:, s, oc, nx : nx + NW],
                                start=False,
                                stop=True,
                            )
                            dst = obufs[oc][:, nx : nx + NW]
                            if eng_flip % 2 == 0:
                                nc.vector.tensor_scalar(
                                    dst,
                                    ps[:, :],
                                    b3_sb[:, oc : oc + 1],
                                    0.0,
                                    op0=ALU.add,
                                    op1=ALU.max,
                                )
                            else:
                                nc.scalar.activation(
                                    dst, ps[:, :], AF.Relu, bias=b3_sb[:, oc : oc + 1]
                                )
                            eng_flip += 1
                for oc in range(2):
                    nc.sync.dma_start(
                        out=out.ap()[s, oc * 128 : (oc + 1) * 128, :],
                        in_=obufs[oc][:, :],
                    )

    nc.compile()
    _CACHE["nc"] = nc
    return nc


def _f32(a):
    return np.ascontiguousarray(a, dtype=np.float32)


def prep_weights(inputs):
    """Host-side folding of BN scales into conv weights; bf16 casts."""
    f = inputs
    s1 = f["bn1_g"] / np.sqrt(f["bn1_v"] + EPS)
    b1_eff = f["bn1_b"] - f["bn1_m"] * s1
    w1t_eff = (_f32(f["conv1_w"]) * s1[:, None]).T          # [256, 64]

    si = f["inv_bn_g"] / np.sqrt(f["inv_bn_v"] + EPS)
    bi_eff = f["inv_bn_b"] - f["inv_bn_m"] * si
    c1t_eff = (_f32(f["inv_c1_w"]) * si[:, None]).T         # [64, 16]

    c2t_eff = _f32(f["inv_c2_w"]).T                         # [16, 196]
    b2c = _f32(f["inv_c2_b"])

    s2 = f["bn2_g"] / np.sqrt(f["bn2_v"] + EPS)
    b2n = f["bn2_b"] - f["bn2_m"] * s2

    s3 = f["bn3_g"] / np.sqrt(f["bn3_v"] + EPS)
    b3_eff = f["bn3_b"] - f["bn3_m"] * s3
    w3t_eff = (_f32(f["conv3_w"]) * s3[:, None]).T          # [64, 256]

    d = {}
    d["w1t"] = np.ascontiguousarray(w1t_eff.reshape(2, 128, CMID).astype(BF16))
    d["b1"] = _f32(b1_eff)[:, None]
    d["c1t"] = np.ascontiguousarray(c1t_eff.astype(BF16))
    d["bi"] = _f32(bi_eff)[:, None]
    d["c2t"] = np.ascontiguousarray(c2t_eff.astype(BF16))
    d["b2ca"] = _f32(b2c[0:128])[:, None]
    d["b2cb"] = _f32(b2c[128:196])[:, None]
    d["s2v"] = _f32(s2)[:, None]
    d["b2v"] = _f32(b2n)[:, None]
    d["w3t"] = np.ascontiguousarray(
        w3t_eff.reshape(CMID, 2, 128).transpose(1, 0, 2).astype(BF16)
    )
    d["b3"] = _f32(b3_eff.reshape(2, 128).T)
    d["ident"] = np.ascontiguousarray(np.eye(128, dtype=np.float32).astype(BF16))
    return d


def make_in_maps(inputs):
    prep = prep_weights(inputs)
    x = _f32(inputs["x"]).reshape(16, CIN, HW)
    in_maps = []
    for i in range(N_CORES):
        m = dict(prep)
        m["xin"] = np.ascontiguousarray(x[S * i : S * i + S])
        in_maps.append(m)
    return in_maps


def kernel(**inputs):
    from concourse.bass_utils import run_bass_kernel_spmd

    nc = build_module()
    in_maps = make_in_maps(inputs)
    res = run_bass_kernel_spmd(nc, in_maps, core_ids=list(range(N_CORES)))
    outs = [
        np.asarray(res.results[i]["out"], dtype=np.float32).reshape(S, CIN, H, W)
        for i in range(N_CORES)
    ]
    return np.concatenate(outs, axis=0).astype(np.float32)
